# revision 3
# baseline (speedup 1.0000x reference)
"""Trainium2 Bass kernel for Transformer-XL style relative-position MHSA.

Strategy: data-parallel over batch (8 batches -> 8 cores). Each core runs the
full module for one batch element. No collectives.

Device layout notes (per core):
  - LayerNorm on x in natural [s, d] layout, output xn cast to bf16.
  - xn transposed once via xbar DMA -> xnT [d, s]; all projections consume xnT.
  - Q/K/P projected "transposed" (d-major): quT/qvT/kT/pT [d', s] with the
    (bq+u)/8 biases and 1/sqrt(hd) scaling folded into the PSUM eviction.
  - V projected natural [s, d] with a ones-column per head appended so the
    attention row-sums (softmax denominators) fall out of the ctx matmul.
  - Relative shift: for each (head, q-tile), pos scores ps[q,k] land in PSUM,
    get evicted bf16 into a staging tile [128, 2048] whose cols [0:1024] hold
    ps[q,:], col 1024 = 0, cols [1025:2048] hold ps[q+1, 0:1023] (filled via a
    partition-shifted SBUF->SBUF DMA). A single DMA with a diagonal access
    pattern (per-partition decreasing offset) then reads the shifted scores:
        shifted[q, k] = staging[q - q0, (S-1-q0) - (q-q0) + k]
    which reproduces jnp.pad+reshape relative_shift exactly.
  - logits = content + shifted_pos (fp32), attn = exp(logits) in bf16
    (softmax max-subtraction skipped: |logits| <~ 15 so exp is safe in fp32).
  - attn transposed per q-tile via xbar DMA -> attnT [k, q]; ctx matmul
    contracts over k with v_aug as the stationary operand, producing
    ctxT [d, q] plus the denominator row; normalization by reciprocal
    of the denominator is applied on eviction.
  - Output projection emits outT [D, s]; host transposes back.
"""

import math
from contextlib import ExitStack

import numpy as np
import ml_dtypes

import concourse.bass as bass
import concourse.bacc as bacc
import concourse.tile as tile
import concourse.mybir as mybir
from concourse import bass_utils

B, S, D, H, HD = 8, 1024, 512, 8, 64
P = 128
NQT = S // P   # 8 q tiles
NKT = S // P   # 8 k tiles
NDT = D // P   # 4 d tiles
NC2 = 2        # 512-wide free-dim chunks per 1024
F32 = mybir.dt.float32
BF16 = mybir.dt.bfloat16
LN_EPS = 1e-5
AX = mybir.AxisListType
ALU = mybir.AluOpType
AF = mybir.ActivationFunctionType


def _sinusoidal_pe() -> np.ndarray:
    pos = np.arange(S, dtype=np.float32)[:, None]
    div = np.exp(
        np.arange(0, D, 2, dtype=np.float32) * (-math.log(10000.0) / D)
    ).astype(np.float32)
    ang = pos * div
    return np.stack([np.sin(ang), np.cos(ang)], axis=-1).reshape(S, D)


def _diag_view(st_ap: bass.AP, qtile: int) -> bass.AP:
    """Diagonal view over a [128, 2048] staging tile:
    elem(dq, k) = staging[dq, (S-1) - 128*qtile - dq + k]."""
    v = st_ap.copy()
    a = v.ap
    a.clear()
    a.extend([(2047, P), (1, S)])
    v.offset = (S - 1) - P * qtile
    return v


def _emit_kernel(ctx: ExitStack, tc: tile.TileContext, io: dict):
    nc = tc.nc
    const = ctx.enter_context(tc.tile_pool(name="const", bufs=1))
    psum = ctx.enter_context(tc.tile_pool(name="psum", bufs=2, space="PSUM"))

    # ---- load weights / constants ----
    w_sb = {}
    for name in ("wq", "wk", "wv", "wo", "wp"):
        tiles = []
        for kt in range(NDT):
            t = const.tile([P, D], BF16, tag=f"{name}{kt}")
            nc.sync.dma_start(t[:], io[name][kt * P:(kt + 1) * P, :])
            tiles.append(t)
        w_sb[name] = tiles
    peT_sb = []
    for kt in range(NDT):
        t = const.tile([P, S], BF16, tag=f"peT{kt}")
        nc.sync.dma_start(t[:], io["peT"][kt * P:(kt + 1) * P, :])
        peT_sb.append(t)
    bias_sb = {}
    for name in ("b_qu", "b_qv", "b_k", "b_o"):
        t = const.tile([P, NDT], F32, tag=name)
        nc.sync.dma_start(t[:], io[name][:])
        bias_sb[name] = t
    bv_sb = const.tile([1, D], F32, tag="b_v")
    nc.sync.dma_start(bv_sb[:], io["b_v"][:])

    # ---- LayerNorm -> xn (bf16), then transpose -> xnT ----
    xnT = const.tile([P, NDT * S], BF16, tag="xnT")  # [do, di*S + s]
    with tc.tile_pool(name="ln", bufs=3) as lnp:
        for st in range(NQT):
            xt = lnp.tile([P, D], F32, tag="xt")
            nc.sync.dma_start(xt[:], io["x"][st * P:(st + 1) * P, :])
            ssum = lnp.tile([P, 1], F32, tag="ssum")
            nc.vector.tensor_reduce(ssum[:], xt[:], AX.X, ALU.add)
            mu = lnp.tile([P, 1], F32, tag="mu")
            nc.vector.tensor_scalar_mul(mu[:], ssum[:], 1.0 / D)
            xc = lnp.tile([P, D], F32, tag="xc")
            nc.vector.tensor_scalar_sub(xc[:], xt[:], mu[:])
            xsq = lnp.tile([P, D], F32, tag="xsq")
            nc.scalar.square(xsq[:], xc[:])
            vsum = lnp.tile([P, 1], F32, tag="vsum")
            nc.vector.tensor_reduce(vsum[:], xsq[:], AX.X, ALU.add)
            varr = lnp.tile([P, 1], F32, tag="varr")
            nc.vector.tensor_scalar(
                varr[:], vsum[:], 1.0 / D, LN_EPS, ALU.mult, ALU.add
            )
            rvar = lnp.tile([P, 1], F32, tag="rvar")
            nc.vector.reciprocal(rvar[:], varr[:])
            rstd = lnp.tile([P, 1], F32, tag="rstd")
            nc.scalar.sqrt(rstd[:], rvar[:])
            xn = lnp.tile([P, D], BF16, tag="xn")
            nc.scalar.activation(xn[:], xc[:], AF.Identity, scale=rstd[:])
            xnT_r = xnT[:].rearrange("p (di s) -> p di s", di=NDT)[
                :, :, st * P:(st + 1) * P
            ]
            nc.sync.dma_start_transpose(out=xnT_r, in_=xn[:])

    # ---- projections: quT/qvT/kT/pT [d', s] ----
    quT = [const.tile([P, S], BF16, tag=f"quT{t}", name=f"quT{t}") for t in range(NDT)]
    qvT = [const.tile([P, S], BF16, tag=f"qvT{t}", name=f"qvT{t}") for t in range(NDT)]
    kT = [const.tile([P, S], BF16, tag=f"kT{t}", name=f"kT{t}") for t in range(NDT)]
    pT = [const.tile([P, S], BF16, tag=f"pT{t}", name=f"pT{t}") for t in range(NDT)]
    for dt in range(NDT):
        for c in range(NC2):
            sl = slice(c * 512, (c + 1) * 512)
            # Q (two evictions: +u and +v biases, both scaled 1/8)
            ps = psum.tile([P, 512], F32, tag="b1")
            for kt in range(NDT):
                nc.tensor.matmul(
                    ps[:],
                    lhsT=w_sb["wq"][kt][:, dt * P:(dt + 1) * P],
                    rhs=xnT[:, kt * S + c * 512: kt * S + (c + 1) * 512],
                    start=(kt == 0), stop=(kt == NDT - 1),
                )
            nc.scalar.activation(
                quT[dt][:, sl], ps[:], AF.Identity,
                bias=bias_sb["b_qu"][:, dt:dt + 1], scale=0.125,
            )
            nc.scalar.activation(
                qvT[dt][:, sl], ps[:], AF.Identity,
                bias=bias_sb["b_qv"][:, dt:dt + 1], scale=0.125,
            )
            # K
            ps = psum.tile([P, 512], F32, tag="b1")
            for kt in range(NDT):
                nc.tensor.matmul(
                    ps[:],
                    lhsT=w_sb["wk"][kt][:, dt * P:(dt + 1) * P],
                    rhs=xnT[:, kt * S + c * 512: kt * S + (c + 1) * 512],
                    start=(kt == 0), stop=(kt == NDT - 1),
                )
            nc.scalar.activation(
                kT[dt][:, sl], ps[:], AF.Identity,
                bias=bias_sb["b_k"][:, dt:dt + 1],
            )
            # P (pos proj, no bias)
            ps = psum.tile([P, 512], F32, tag="b1")
            for kt in range(NDT):
                nc.tensor.matmul(
                    ps[:],
                    lhsT=w_sb["wp"][kt][:, dt * P:(dt + 1) * P],
                    rhs=peT_sb[kt][:, c * 512:(c + 1) * 512],
                    start=(kt == 0), stop=(kt == NDT - 1),
                )
            nc.scalar.activation(pT[dt][:, sl], ps[:], AF.Copy)

    # ---- V natural [s, d]; bv added via a rank-1 (K=1) matmul accumulate ----
    ones1 = const.tile([1, P], BF16, tag="ones1")
    nc.gpsimd.memset(ones1[:], 1.0)
    bv_bf = const.tile([1, D], BF16, tag="bv_bf")
    nc.vector.tensor_copy(bv_bf[:], bv_sb[:])
    v_sb = [const.tile([P, D], BF16, tag=f"vsb{st}", name=f"vsb{st}") for st in range(NQT)]
    for st in range(NQT):
        ps = psum.tile([P, 512], F32, tag="b1")
        for kt in range(NDT):
            nc.tensor.matmul(
                ps[:],
                lhsT=xnT[:, kt * S + st * P: kt * S + st * P + P],
                rhs=w_sb["wv"][kt][:],
                start=(kt == 0), stop=False,
            )
        nc.tensor.matmul(ps[:], lhsT=ones1[:], rhs=bv_bf[:], start=False, stop=True)
        nc.scalar.activation(v_sb[st][:], ps[:], AF.Copy)

    # ---- main attention loop ----
    stg_pool = ctx.enter_context(tc.tile_pool(name="stg", bufs=4))
    ep_pool = ctx.enter_context(tc.tile_pool(name="ep", bufs=2))
    lg_pool = ctx.enter_context(tc.tile_pool(name="lg", bufs=2))
    at_pool = ctx.enter_context(tc.tile_pool(name="at", bufs=2))
    atT_pool = ctx.enter_context(tc.tile_pool(name="atT", bufs=2))
    cx_pool = ctx.enter_context(tc.tile_pool(name="cx", bufs=2))
    ctxT_all = [const.tile([P, S], BF16, tag=f"ctxT{t}", name=f"ctxT{t}") for t in range(NDT)]

    for h in range(H):
        dt_h, off = divmod(h * HD, P)
        hsl = slice(off, off + HD)
        attnT = atT_pool.tile([P, NKT * P * NQT], BF16, tag="attnT")  # [128, 8192]

        def make_staging(I):
            psA = psum.tile([P, S], F32, tag="psA")
            for c in range(NC2):
                nc.tensor.matmul(
                    psA[:, c * 512:(c + 1) * 512],
                    lhsT=qvT[dt_h][hsl, I * P:(I + 1) * P],
                    rhs=pT[dt_h][hsl, c * 512:(c + 1) * 512],
                    start=True, stop=True,
                )
            st_t = stg_pool.tile([P, 2 * S], BF16, tag="stg")
            nc.scalar.activation(st_t[:, 0:S], psA[:], AF.Copy)
            nc.gpsimd.memset(st_t[:, S:S + 1], 0.0)
            return st_t

        def process_tile(I, st_t, st_next):
            # fill shifted region: staging[dq, 1025+j] = ps[q+1, j]
            nc.sync.dma_start(
                out=st_t[0:P - 1, S + 1:2 * S], in_=st_t[1:P, 0:S - 1]
            )
            if st_next is not None:
                nc.sync.dma_start(
                    out=st_t[P - 1:P, S + 1:2 * S], in_=st_next[0:1, 0:S - 1]
                )
            ep = ep_pool.tile([P, S], BF16, tag="ep")
            nc.sync.dma_start(out=ep[:], in_=_diag_view(st_t[:], I))
            # content scores
            logit = lg_pool.tile([P, S], F32, tag="logit")
            for c in range(NC2):
                sl = slice(c * 512, (c + 1) * 512)
                psC = psum.tile([P, 512], F32, tag="psC")
                nc.tensor.matmul(
                    psC[:],
                    lhsT=quT[dt_h][hsl, I * P:(I + 1) * P],
                    rhs=kT[dt_h][hsl, sl],
                    start=True, stop=True,
                )
                nc.vector.tensor_add(logit[:, sl], psC[:], ep[:, sl])
            attn = at_pool.tile([P, S], BF16, tag="attn")
            sums = at_pool.tile([P, 1], F32, tag="sums")
            nc.scalar.activation(attn[:], logit[:], AF.Exp, accum_out=sums[:])
            recip = at_pool.tile([P, 1], F32, tag="recip")
            nc.vector.reciprocal(recip[:], sums[:])
            nc.vector.tensor_scalar_mul(attn[:], attn[:], recip[:])
            attnT_r = attnT[:].rearrange("p (di s) -> p di s", di=NKT)[
                :, :, I * P:(I + 1) * P
            ]
            nc.sync.dma_start_transpose(out=attnT_r, in_=attn[:])

        prev = None
        for I in range(NQT):
            st_t = make_staging(I)
            if prev is not None:
                process_tile(I - 1, prev, st_t)
            prev = st_t
        process_tile(NQT - 1, prev, None)

        # ctx matmul: ctxT[d, q] (+ denominator row) then normalize
        for c in range(NC2):
            sl = slice(c * 512, (c + 1) * 512)
            cps = psum.tile([HD, 512], F32, tag="b1")
            for kt in range(NKT):
                nc.tensor.matmul(
                    cps[:],
                    lhsT=v_sb[kt][:, h * HD:(h + 1) * HD],
                    rhs=attnT[:, kt * S + c * 512: kt * S + (c + 1) * 512],
                    start=(kt == 0), stop=(kt == NKT - 1),
                )
            ctxn = cx_pool.tile([HD, 512], BF16, tag="ctxn")
            nc.scalar.activation(ctxn[:], cps[:], AF.Copy)
            nc.sync.dma_start(out=ctxT_all[dt_h][hsl, sl], in_=ctxn[:])

    # ---- output projection: outT[dD, s] = Wo^T @ ctxT (+bo) ----
    with tc.tile_pool(name="outp", bufs=2) as outp:
        for dt in range(NDT):
            ot = outp.tile([P, S], F32, tag="ot")
            for c in range(NC2):
                ps = psum.tile([P, 512], F32, tag="b1")
                for kt in range(NDT):
                    nc.tensor.matmul(
                        ps[:],
                        lhsT=w_sb["wo"][kt][:, dt * P:(dt + 1) * P],
                        rhs=ctxT_all[kt][:, c * 512:(c + 1) * 512],
                        start=(kt == 0), stop=(kt == NDT - 1),
                    )
                nc.scalar.activation(
                    ot[:, c * 512:(c + 1) * 512], ps[:], AF.Identity,
                    bias=bias_sb["b_o"][:, dt:dt + 1],
                )
            nc.sync.dma_start(io["outT"][dt * P:(dt + 1) * P, :], ot[:])


_PROGRAM_CACHE = {}


def _get_program():
    if "nc" in _PROGRAM_CACHE:
        return _PROGRAM_CACHE["nc"]
    nc = bacc.Bacc("TRN2", target_bir_lowering=False, debug=False)
    io = {}
    io["x"] = nc.dram_tensor("x", [S, D], F32, kind="ExternalInput")
    for name in ("wq", "wk", "wv", "wo", "wp"):
        io[name] = nc.dram_tensor(name, [D, D], BF16, kind="ExternalInput")
    io["peT"] = nc.dram_tensor("peT", [D, S], BF16, kind="ExternalInput")
    for name in ("b_qu", "b_qv", "b_k", "b_o"):
        io[name] = nc.dram_tensor(name, [P, NDT], F32, kind="ExternalInput")
    io["b_v"] = nc.dram_tensor("b_v", [1, D], F32, kind="ExternalInput")
    io["outT"] = nc.dram_tensor("outT", [D, S], F32, kind="ExternalOutput")
    with tile.TileContext(nc) as tc:
        with ExitStack() as ctx:
            _emit_kernel(ctx, tc, io)
    nc.compile()
    _PROGRAM_CACHE["nc"] = nc
    return nc


def make_in_maps(**inputs) -> list[dict]:
    x = np.asarray(inputs["x"], np.float32)
    g = np.asarray(inputs["ln_g"], np.float32)
    bln = np.asarray(inputs["ln_b"], np.float32)
    Wq = np.asarray(inputs["Wq"], np.float32)
    Wk = np.asarray(inputs["Wk"], np.float32)
    Wv = np.asarray(inputs["Wv"], np.float32)
    Wo = np.asarray(inputs["Wo"], np.float32)
    Wp = np.asarray(inputs["Wp"], np.float32)
    bq = np.asarray(inputs["bq"], np.float32)
    bk = np.asarray(inputs["bk"], np.float32)
    bv = np.asarray(inputs["bv"], np.float32)
    bo = np.asarray(inputs["bo"], np.float32)
    u = np.asarray(inputs["u_bias"], np.float32).reshape(-1)
    v = np.asarray(inputs["v_bias"], np.float32).reshape(-1)

    bf = ml_dtypes.bfloat16
    # fold LN affine (gamma/beta) into the projections that consume xn
    Wq_, Wk_, Wv_ = g[:, None] * Wq, g[:, None] * Wk, g[:, None] * Wv
    bq_, bk_, bv_ = bln @ Wq + bq, bln @ Wk + bk, bln @ Wv + bv
    pe = _sinusoidal_pe()

    def pcol(vec):  # [D] -> [P, NDT] per-partition bias layout
        return np.ascontiguousarray(vec.reshape(NDT, P).T.astype(np.float32))

    common = {
        "wq": Wq_.astype(bf), "wk": Wk_.astype(bf), "wv": Wv_.astype(bf),
        "wo": Wo.astype(bf), "wp": Wp.astype(bf),
        "peT": np.ascontiguousarray(pe.T).astype(bf),
        "b_qu": pcol((bq_ + u) / 8.0), "b_qv": pcol((bq_ + v) / 8.0),
        "b_k": pcol(bk_), "b_o": pcol(bo),
        "b_v": bv_.reshape(1, D).astype(np.float32),
    }
    return [dict(common, x=np.ascontiguousarray(x[b])) for b in range(B)]


def kernel(**inputs) -> np.ndarray:
    nc = _get_program()
    in_maps = make_in_maps(**inputs)
    res = bass_utils.run_bass_kernel_spmd(nc, in_maps, list(range(B)))
    out = np.stack([np.asarray(res.results[b]["outT"]).T for b in range(B)])
    return np.ascontiguousarray(out.astype(np.float32))
